# revision 1
# baseline (speedup 1.0000x reference)
"""Trainium2 Bass kernel for a 6-layer GPT-style decoder.

Model: L=6, E=384, S=256, H=6, D=64, V=65, FF=1536, batch B=64.
Sharding: data-parallel over batch across 8 NeuronCores (8 sequences =
2048 tokens per core); all ~10M params replicated per core. No
collectives needed; host gathers per-core logits.

Per-core kernel layout notes:
  - tokens live on partitions in 16 tiles of [128, E]
  - LN is computed in natural [token, E] layout (bn_stats/bn_aggr), the
    normalized activations are transposed on the PE (via identity
    matmul) into [E, token] layout which feeds every matmul as lhsT/rhs
    without further transposes:
      qT/kT  = Wq/Wk (lhsT, natural) x hnT   -> [head_dim, tok]
      v      = hnT (lhsT) x Wv (natural)     -> [tok, head_dim]
      scores = qT (lhsT) x kT                -> [q, k] (+causal mask, softmax)
      attT   = v (lhsT) x P^T                -> [head_dim, tok]
      proj   = attT (lhsT) x Wproj (natural) -> [tok, E] (+residual)
      ffT    = W1 (lhsT) x hn2T              -> [ff, tok] (relu+b1 fused)
      ffn    = ffT (lhsT) x W2 (natural)     -> [tok, E] (+residual)
      logitsT= Wlm (lhsT) x hfT              -> [V, tok]
  - matmuls run in float32r (full fp32 data, reduced-precision PE mode,
    1 cycle/row when the moving dim >= 256 vs 4 for fp32)
  - causal mask is additive (0 / -1e9) and precomputed host-side
  - softmax skips max-subtraction: scores are O(1) for this model scale
    (LN'd activations times 0.02-std weights), exp cannot overflow
"""

import sys
import numpy as np

for _p in ("/opt/trn_rl_repo",):
    if _p not in sys.path:
        sys.path.insert(0, _p)

import concourse.bass as bass
import concourse.bacc as bacc
import concourse.mybir as mybir
import concourse.tile as tile
from concourse.masks import make_identity

F32 = mybir.dt.float32
F32R = mybir.dt.float32r
# matmul operand dtype: float32r = full fp32 data, reduced-precision PE
# mode at 1 cycle/row (vs 4 for fp32) when the moving dim >= 256
MMD = F32R
AF = mybir.ActivationFunctionType
ALU = mybir.AluOpType

L, E, S, NH, DH, V = 6, 384, 256, 6, 64, 65
FF = 4 * E
B = 64
NCORES = 8
BPC = B // NCORES           # sequences per core
TOK = BPC * S               # 2048 tokens per core
NTT = TOK // 128            # 16 token tiles
NET = E // 128              # 3 E tiles
NFT = FF // 128             # 12 FF tiles
NPAIR = TOK // 512          # 4 pairs of sequences (512-token chunks)
NEG = -1.0e9
EPS = 1e-5

_cache = {}


def _build_program(use_ln_g, use_ln_b, use_bproj, use_b2, use_lnf_g, use_lnf_b):
    """Build the SPMD Bass program (identical on all 8 cores)."""
    from contextlib import ExitStack

    nc = bacc.Bacc("TRN2", target_bir_lowering=False, debug=False)

    h0_d = nc.dram_tensor("h0", [TOK, E], F32, kind="ExternalInput")
    wq_d = nc.dram_tensor("wq", [L, E, NH * DH], MMD, kind="ExternalInput")
    wk_d = nc.dram_tensor("wk", [L, E, NH * DH], MMD, kind="ExternalInput")
    wv_d = nc.dram_tensor("wv", [L, E, NH * DH], MMD, kind="ExternalInput")
    wproj_d = nc.dram_tensor("wproj", [L, NH * DH, E], MMD, kind="ExternalInput")
    bproj_d = nc.dram_tensor("bproj", [L, E], F32, kind="ExternalInput")
    w1_d = nc.dram_tensor("w1", [L, E, FF], MMD, kind="ExternalInput")
    b1_d = nc.dram_tensor("b1", [L, FF], F32, kind="ExternalInput")
    w2_d = nc.dram_tensor("w2", [L, FF, E], MMD, kind="ExternalInput")
    b2_d = nc.dram_tensor("b2", [L, E], F32, kind="ExternalInput")
    ln1g_d = nc.dram_tensor("ln1_g", [L, E], F32, kind="ExternalInput")
    ln1b_d = nc.dram_tensor("ln1_b", [L, E], F32, kind="ExternalInput")
    ln2g_d = nc.dram_tensor("ln2_g", [L, E], F32, kind="ExternalInput")
    ln2b_d = nc.dram_tensor("ln2_b", [L, E], F32, kind="ExternalInput")
    lnfg_d = nc.dram_tensor("lnf_g", [E], F32, kind="ExternalInput")
    lnfb_d = nc.dram_tensor("lnf_b", [E], F32, kind="ExternalInput")
    wlm_d = nc.dram_tensor("wlm", [E, V], MMD, kind="ExternalInput")
    blm_d = nc.dram_tensor("blm", [V], F32, kind="ExternalInput")
    mask_d = nc.dram_tensor("mask", [2, 128, 2 * 128], F32, kind="ExternalInput")
    ones_d = nc.dram_tensor("ones", [128, 64], MMD, kind="ExternalInput")
    out_d = nc.dram_tensor("logitsT", [V, TOK], F32, kind="ExternalOutput")

    def bcast_row(dram_ap, p=128):
        # replicate a [N] dram vector across p partitions
        return bass.AP(
            tensor=dram_ap.tensor,
            offset=dram_ap.offset,
            ap=[[0, p]] + list(dram_ap.ap),
        )

    with tile.TileContext(nc) as tc, ExitStack() as ctx:
        const = ctx.enter_context(tc.tile_pool(name="const", bufs=1))
        hpool = ctx.enter_context(tc.tile_pool(name="hpool", bufs=16))
        wpool = ctx.enter_context(tc.tile_pool(name="wpool", bufs=1))
        hTpool = ctx.enter_context(tc.tile_pool(name="hTpool", bufs=2))
        hnpool = ctx.enter_context(tc.tile_pool(name="hnpool", bufs=3))
        qkpool = ctx.enter_context(tc.tile_pool(name="qkpool", bufs=2))
        appool = ctx.enter_context(tc.tile_pool(name="appool", bufs=2))
        ffpool = ctx.enter_context(tc.tile_pool(name="ffpool", bufs=2))
        stats = ctx.enter_context(tc.tile_pool(name="stats", bufs=8))
        opool = ctx.enter_context(tc.tile_pool(name="opool", bufs=2))

        ps_mm = ctx.enter_context(tc.tile_pool(name="ps_mm", bufs=2, space="PSUM"))
        ps_s = ctx.enter_context(tc.tile_pool(name="ps_s", bufs=2, space="PSUM"))
        ps_a = ctx.enter_context(tc.tile_pool(name="ps_a", bufs=4, space="PSUM"))

        ident = const.tile([128, 128], F32)
        make_identity(nc, ident)
        masks = const.tile([128, 2, 256], F32)
        nc.gpsimd.dma_start(masks, mask_d.rearrange("q p k -> p q k"))
        epst = const.tile([128, 1], F32)
        nc.vector.memset(epst, EPS)
        ones_sb = const.tile([128, 64], MMD)
        nc.gpsimd.dma_start(ones_sb, ones_d.ap())

        wlm_sb = const.tile([128, NET, V], MMD)
        nc.gpsimd.dma_start(wlm_sb, wlm_d.rearrange("(et p) v -> p et v", p=128))
        blm_sb = const.tile([V, 1], F32)
        blm_ap = blm_d.ap()
        nc.gpsimd.dma_start(
            blm_sb,
            bass.AP(
                tensor=blm_ap.tensor,
                offset=blm_ap.offset,
                ap=list(blm_ap.ap) + [[1, 1]],
            ),
        )
        if use_lnf_g:
            lnfg_sb = const.tile([128, E], F32)
            nc.gpsimd.dma_start(lnfg_sb, bcast_row(lnfg_d))
        if use_lnf_b:
            lnfb_sb = const.tile([128, E], F32)
            nc.gpsimd.dma_start(lnfb_sb, bcast_row(lnfb_d))

        # persistent residual stream: 16 tiles of [128, E]
        h = [
            hpool.tile([128, E], F32, tag="h", name=f"h{i}") for i in range(NTT)
        ]
        h0_r = h0_d.rearrange("(tt p) e -> p tt e", p=128)
        for t in range(NTT):
            nc.gpsimd.dma_start(h[t], h0_r[:, t, :])

        def layer_norm(out_aps, x_aps, corr, g_sb, b_sb):
            # batched across tiles: one sqrt + one reciprocal per batch
            # instead of per tile, collapsing the DVE->ACT->DVE ping-pong
            n = len(x_aps)
            st = stats.tile([128, n, 6], F32, tag="st")
            for i, x in enumerate(x_aps):
                nc.vector.bn_stats(st[:, i, :], x)
            mv = stats.tile([128, n, 2], F32, tag="mv")
            for i in range(n):
                nc.vector.bn_aggr(mv[:, i, :], st[:, i, :])
            sd = stats.tile([128, n], F32, tag="sd")
            # sd = sqrt(var * corr + eps)
            nc.scalar.activation(sd, mv[:, :, 1], AF.Sqrt, bias=epst, scale=corr)
            rstd = stats.tile([128, n], F32, tag="rstd")
            nc.vector.reciprocal(rstd, sd)
            for i, (o, x) in enumerate(zip(out_aps, x_aps)):
                nc.gpsimd.tensor_scalar(
                    o, x, mv[:, i, 0:1], rstd[:, i : i + 1],
                    op0=ALU.subtract, op1=ALU.mult,
                )
                if g_sb is not None:
                    nc.vector.tensor_mul(o, o, g_sb)
                if b_sb is not None:
                    nc.vector.tensor_add(o, o, b_sb)

        def transpose_128(dst_ap, src_ap):
            # dst[128, 128] = src[128, 128]^T via PE; copy PSUM->SBUF on ACT
            pst = ps_s.tile([128, 128], F32, tag="ss", name="pst")
            nc.tensor.transpose(pst, src_ap, ident)
            nc.scalar.copy(dst_ap, pst)

        for l in range(L):
            wq_sb = wpool.tile([128, NET, NH * DH], MMD, tag="wq")
            nc.gpsimd.dma_start(wq_sb, wq_d[l].rearrange("(et p) n -> p et n", p=128))
            wk_sb = wpool.tile([128, NET, NH * DH], MMD, tag="wk")
            nc.gpsimd.dma_start(wk_sb, wk_d[l].rearrange("(et p) n -> p et n", p=128))
            wv_sb = wpool.tile([128, NET, NH * DH], MMD, tag="wv")
            nc.gpsimd.dma_start(wv_sb, wv_d[l].rearrange("(et p) n -> p et n", p=128))
            wp_sb = wpool.tile([128, NET, E], MMD, tag="wp")
            nc.gpsimd.dma_start(wp_sb, wproj_d[l].rearrange("(dt p) e -> p dt e", p=128))
            w1_sb = wpool.tile([128, NET, FF], MMD, tag="w1")
            nc.gpsimd.dma_start(w1_sb, w1_d[l].rearrange("(et p) f -> p et f", p=128))
            w2_sb = wpool.tile([128, NFT, E], MMD, tag="w2")
            nc.gpsimd.dma_start(w2_sb, w2_d[l].rearrange("(ft p) e -> p ft e", p=128))
            b1_sb = wpool.tile([128, NFT], F32, tag="b1")
            nc.gpsimd.dma_start(b1_sb, b1_d[l].rearrange("(ft p) -> p ft", p=128))

            g1_sb = b1t_sb = g2_sb = b2t_sb = None
            if use_ln_g:
                g1_sb = wpool.tile([128, E], F32, tag="g1")
                nc.gpsimd.dma_start(g1_sb, bcast_row(ln1g_d[l]))
                g2_sb = wpool.tile([128, E], F32, tag="g2")
                nc.gpsimd.dma_start(g2_sb, bcast_row(ln2g_d[l]))
            if use_ln_b:
                b1t_sb = wpool.tile([128, E], F32, tag="lb1")
                nc.gpsimd.dma_start(b1t_sb, bcast_row(ln1b_d[l]))
                b2t_sb = wpool.tile([128, E], F32, tag="lb2")
                nc.gpsimd.dma_start(b2t_sb, bcast_row(ln2b_d[l]))
            bp_sb = None
            if use_bproj:
                bp_sb = wpool.tile([128, E], F32, tag="bp")
                nc.gpsimd.dma_start(bp_sb, bcast_row(bproj_d[l]))
            b2r_sb = None
            if use_b2:
                b2r_sb = wpool.tile([128, E], F32, tag="b2r")
                nc.gpsimd.dma_start(b2r_sb, bcast_row(b2_d[l]))

            # ---- attention, one 512-token pair of sequences at a time ----
            for p in range(NPAIR):
                hnT = hTpool.tile([128, NET, 512], MMD, tag="hT")
                hns = [
                    hnpool.tile([128, E], F32, tag="hn", bufs=4, name=f"hn{i}")
                    for i in range(4)
                ]
                layer_norm(
                    hns, [h[4 * p + t4] for t4 in range(4)],
                    float(E) / (E - 1), g1_sb, b1t_sb,
                )
                for t4 in range(4):
                    for et in range(NET):
                        transpose_128(
                            hnT[:, et, 128 * t4 : 128 * (t4 + 1)],
                            hns[t4][:, 128 * et : 128 * (et + 1)],
                        )

                qT = qkpool.tile([128, NET, 512], MMD, tag="qT")
                kT = qkpool.tile([128, NET, 512], MMD, tag="kT")
                for dt_ in range(NET):
                    psq = ps_mm.tile([128, 512], F32, tag="mm")
                    for et in range(NET):
                        nc.tensor.matmul(
                            psq,
                            wq_sb[:, et, 128 * dt_ : 128 * (dt_ + 1)],
                            hnT[:, et, :],
                            start=(et == 0),
                            stop=(et == NET - 1),
                        )
                    # fold the 1/sqrt(DH) softmax scale into q
                    nc.vector.tensor_scalar_mul(qT[:, dt_, :], psq, DH**-0.5)
                    psk = ps_mm.tile([128, 512], F32, tag="mm")
                    for et in range(NET):
                        nc.tensor.matmul(
                            psk,
                            wk_sb[:, et, 128 * dt_ : 128 * (dt_ + 1)],
                            hnT[:, et, :],
                            start=(et == 0),
                            stop=(et == NET - 1),
                        )
                    nc.vector.tensor_copy(kT[:, dt_, :], psk)

                v_sb = qkpool.tile([128, 4, NH * DH], MMD, tag="v")
                for t4 in range(4):
                    psv = ps_mm.tile([128, NH * DH], F32, tag="mm")
                    for et in range(NET):
                        nc.tensor.matmul(
                            psv,
                            hnT[:, et, 128 * t4 : 128 * (t4 + 1)],
                            wv_sb[:, et, :],
                            start=(et == 0),
                            stop=(et == NET - 1),
                        )
                    nc.scalar.copy(v_sb[:, t4, :], psv)

                attT = appool.tile([128, NET, 512], MMD, tag="attT")
                for s_ in range(2):
                    kc = 256 * s_
                    for hh in range(NH):
                        pr = (hh % 2) * 64
                        dt_ = hh // 2
                        hc = hh * DH
                        # scores computed TRANSPOSED: S^T[k, q] = kT.T @ qT,
                        # so exp(S^T) feeds the PV matmul directly (no
                        # P-transposes). Softmax denominator comes from a
                        # ones-matmul over exp(S^T); normalization is applied
                        # to the PV output (recip + mul).
                        pss = ps_s.tile([128, 2, 256], F32, tag="ss")
                        for kt in range(2):
                            nc.tensor.matmul(
                                pss[:, kt, :],
                                kT[pr : pr + 64, dt_, kc + 128 * kt : kc + 128 * (kt + 1)],
                                qT[pr : pr + 64, dt_, kc : kc + 256],
                                start=True,
                                stop=True,
                            )
                        # causal mask in [k, q] layout: k-tile 0 only masks
                        # its q < 128 diagonal block; k-tile 1 masks everywhere
                        nc.vector.tensor_add(
                            pss[:, 0, 0:128], pss[:, 0, 0:128], masks[:, 0, 0:128]
                        )
                        nc.vector.tensor_add(pss[:, 1, :], pss[:, 1, :], masks[:, 1, :])
                        PT = appool.tile([128, 2, 256], MMD, tag="PT", bufs=4)
                        nc.scalar.activation(PT[:, 0, :], pss[:, 0, :], AF.Exp)
                        nc.scalar.activation(PT[:, 1, :], pss[:, 1, :], AF.Exp)
                        psz = ps_a.tile([64, 256], F32, tag="pz", name="psz")
                        psa = ps_a.tile([64, 256], F32, tag="pz", name="psa")
                        for kt in range(2):
                            nc.tensor.matmul(
                                psz,
                                ones_sb,
                                PT[:, kt, :],
                                start=(kt == 0),
                                stop=(kt == 1),
                            )
                            nc.tensor.matmul(
                                psa,
                                v_sb[:, 2 * s_ + kt, hc : hc + DH],
                                PT[:, kt, :],
                                start=(kt == 0),
                                stop=(kt == 1),
                            )
                        zr = appool.tile([64, 256], F32, tag="zr", bufs=4)
                        nc.vector.reciprocal(zr, psz)
                        nc.vector.tensor_mul(
                            attT[pr : pr + 64, dt_, kc : kc + 256], psa, zr
                        )

                for t4 in range(4):
                    t = 4 * p + t4
                    psp = ps_mm.tile([128, E], F32, tag="mm")
                    for dt_ in range(NET):
                        nc.tensor.matmul(
                            psp,
                            attT[:, dt_, 128 * t4 : 128 * (t4 + 1)],
                            wp_sb[:, dt_, :],
                            start=(dt_ == 0),
                            stop=(dt_ == NET - 1),
                        )
                    nc.vector.tensor_add(h[t], h[t], psp)
                    if bp_sb is not None:
                        nc.vector.tensor_add(h[t], h[t], bp_sb)

            # ---- FFN, one 256-token chunk at a time ----
            for c in range(NTT // 2):
                h2T = hTpool.tile([128, NET, 256], MMD, tag="h2T")
                hns = [
                    hnpool.tile([128, E], F32, tag="hn", bufs=4, name=f"hn2_{i}")
                    for i in range(2)
                ]
                layer_norm(
                    hns, [h[2 * c + t2] for t2 in range(2)],
                    float(E) / (E - 1), g2_sb, b2t_sb,
                )
                for t2 in range(2):
                    for et in range(NET):
                        transpose_128(
                            h2T[:, et, 128 * t2 : 128 * (t2 + 1)],
                            hns[t2][:, 128 * et : 128 * (et + 1)],
                        )
                ffT = ffpool.tile([128, NFT, 256], MMD, tag="ffT")
                for ft in range(NFT):
                    psf = ps_mm.tile([128, 256], F32, tag="mm")
                    for et in range(NET):
                        nc.tensor.matmul(
                            psf,
                            w1_sb[:, et, 128 * ft : 128 * (ft + 1)],
                            h2T[:, et, :],
                            start=(et == 0),
                            stop=(et == NET - 1),
                        )
                    nc.scalar.activation(
                        ffT[:, ft, :], psf, AF.Relu, bias=b1_sb[:, ft : ft + 1]
                    )
                for t2 in range(2):
                    t = 2 * c + t2
                    psw = ps_mm.tile([128, E], F32, tag="mm")
                    for ft in range(NFT):
                        nc.tensor.matmul(
                            psw,
                            ffT[:, ft, 128 * t2 : 128 * (t2 + 1)],
                            w2_sb[:, ft, :],
                            start=(ft == 0),
                            stop=(ft == NFT - 1),
                        )
                    nc.vector.tensor_add(h[t], h[t], psw)
                    if b2r_sb is not None:
                        nc.vector.tensor_add(h[t], h[t], b2r_sb)

        # ---- final LN + LM head ----
        for p in range(NPAIR):
            hfT = hTpool.tile([128, NET, 512], MMD, tag="hT")
            hns = [
                hnpool.tile([128, E], F32, tag="hn", bufs=4, name=f"hnf{i}")
                for i in range(4)
            ]
            layer_norm(
                hns, [h[4 * p + t4] for t4 in range(4)], 1.0,
                lnfg_sb if use_lnf_g else None,
                lnfb_sb if use_lnf_b else None,
            )
            for t4 in range(4):
                for et in range(NET):
                    transpose_128(
                        hfT[:, et, 128 * t4 : 128 * (t4 + 1)],
                        hns[t4][:, 128 * et : 128 * (et + 1)],
                    )
            pso = ps_mm.tile([V, 512], F32, tag="mm")
            for et in range(NET):
                nc.tensor.matmul(
                    pso,
                    wlm_sb[:, et, :],
                    hfT[:, et, :],
                    start=(et == 0),
                    stop=(et == NET - 1),
                )
            lo = opool.tile([V, 512], F32, tag="lo")
            nc.vector.tensor_scalar_add(lo, pso, blm_sb)
            nc.sync.dma_start(out_d[:, 512 * p : 512 * (p + 1)], lo)

    nc.compile()
    return nc


def _get_program(flags):
    if flags not in _cache:
        _cache[flags] = _build_program(*flags)
    return _cache[flags]


def kernel(**inputs):
    x = np.asarray(inputs["x"])
    tok_emb = np.asarray(inputs["tok_emb"], dtype=np.float32)
    pos_emb = np.asarray(inputs["pos_emb"], dtype=np.float32)

    f32 = lambda k: np.ascontiguousarray(np.asarray(inputs[k], dtype=np.float32))
    wq, wk, wv = f32("Wq"), f32("Wk"), f32("Wv")
    wproj, bproj = f32("Wproj"), f32("bproj")
    w1, b1, w2, b2 = f32("W1"), f32("b1"), f32("W2"), f32("b2")
    ln1g, ln1b = f32("ln1_g"), f32("ln1_b")
    ln2g, ln2b = f32("ln2_g"), f32("ln2_b")
    lnfg, lnfb = f32("lnf_g"), f32("lnf_b")
    wlm, blm = f32("Wlm"), f32("blm")

    flags = (
        bool(np.any(ln1g != 1.0) or np.any(ln2g != 1.0)),
        bool(np.any(ln1b != 0.0) or np.any(ln2b != 0.0)),
        bool(np.any(bproj != 0.0)),
        bool(np.any(b2 != 0.0)),
        bool(np.any(lnfg != 1.0)),
        bool(np.any(lnfb != 0.0)),
    )
    nc = _get_program(flags)

    # host-side embedding gather (tiny) + batch sharding
    T = x.shape[1]
    emb = tok_emb[x] + pos_emb[:T][None, :, :]  # [B, S, E] f32
    emb = emb.astype(np.float32)

    # additive causal mask in transposed [k, q] layout
    mask = np.zeros((2, 128, 256), dtype=np.float32)
    for kt in range(2):
        k = kt * 128 + np.arange(128)[:, None]
        q = np.arange(256)[None, :]
        mask[kt][k > q] = NEG
    ones = np.ones((128, 64), dtype=np.float32)

    shared = dict(
        wq=wq, wk=wk, wv=wv, wproj=wproj, bproj=bproj,
        w1=w1, b1=b1, w2=w2, b2=b2,
        ln1_g=ln1g, ln1_b=ln1b, ln2_g=ln2g, ln2_b=ln2b,
        lnf_g=lnfg, lnf_b=lnfb, wlm=wlm, blm=blm, mask=mask, ones=ones,
    )
    in_maps = []
    for c in range(NCORES):
        h0 = np.ascontiguousarray(
            emb[c * BPC : (c + 1) * BPC].reshape(TOK, E)
        )
        in_maps.append(dict(shared, h0=h0))

    from concourse.bass_utils import run_bass_kernel_spmd

    res = run_bass_kernel_spmd(nc, in_maps, list(range(NCORES)))
    outs = []
    for c in range(NCORES):
        lt = res.results[c]["logitsT"]  # [V, TOK]
        outs.append(lt.T.reshape(BPC, S, V))
    return np.concatenate(outs, axis=0).astype(np.float32)

